# revision 1
# baseline (speedup 1.0000x reference)
"""BilateralRotation Trainium2 kernel: out[b,c] = R1[c] @ wkv[b,c] @ R2[c],
R = Cayley(p) = (I - A)(I + A)^-1, A = 0.5(p - p^T).

Sharding: 8 NeuronCores, head-parallel — core k owns heads [4k, 4k+4) for all
512 batches (32 MB in / 32 MB out per core; the tiny per-head rotations are
computed on-device per core via Newton-Schulz).

Device program per core:
  Phase 1 — Cayley on-device: B = A^T A (PE), M = I + B (SPD),
    Newton-Schulz X' = 2X - X(M X) with X0 = I/300 -> X = M^-1 (15 iters,
    fp32; X^T tracked via exact PE transposes so skew error cancels),
    R = C X with C = I - 2A - B.
  Phase 2 — bilateral rotation, fully pipelined:
    - input DMA in a folded layout (each partition holds two consecutive
      h-rows = 512B contiguous elements; 128 partitions span the core's 4
      heads with a single affine stride) -> full DMA bandwidth
    - MM1 (Y = R1 X): two parity-split accumulating matmuls per head-pair,
      stationary = blockdiag of parity-sliced R1^T, moving = data, float32r
    - T1: PE 128x128 transposes (4 items each)
    - MM2 (Z^T = R2^T Y^T): stationary blockdiag(R2,R2), strided rhs
      gathering one head's columns, N=512, float32r
    - Z^T tiles dumped contiguously to HBM; the host inverts the (fixed,
      known) index permutation while unsharding/concatenating the 8 shards.
"""

import sys
import types
from contextlib import ExitStack

import numpy as np

# ---------------------------------------------------------------------------
# TileContext patch: this walrus build accepts only ONE sync-wait per
# instruction; hoist extra waits onto nops inserted before the instruction.
# ---------------------------------------------------------------------------
import concourse.bass as bass
import concourse.tile as tile
from concourse.vector_clock import ScopedClock
from concourse import masks, mybir
from concourse.bass_utils import run_bass_kernel_spmd

WAIT_LIMIT = 1


def _hoist_extra_waits(nc, inst, hint):
    nops = []
    si = inst.sync_info
    if si is not None and len(si.on_wait) > WAIT_LIMIT:
        extras = si.on_wait[:-WAIT_LIMIT]
        del si.on_wait[:-WAIT_LIMIT]
        for w in extras:
            nop = nc.engines[inst.engine].nop(nofuse=True, hint=hint)
            nsi = nop.ins.sync_info
            if nsi is None:
                nop.ins.sync_info = mybir.SyncInfo(on_wait=[w], on_update=[])
            else:
                nsi.on_wait.append(w)
            nops.append(nop.ins)
    return nops


def _split_waits(nc):
    cur_list = nc.cur_bb.bb.instructions
    for f in nc.m.functions:
        for bb in f.blocks:
            orig = list(bb.instructions)
            if not any(i.sync_info and len(i.sync_info.on_wait) > WAIT_LIMIT
                       for i in orig):
                continue
            new_list = []
            for inst in orig:
                nops = _hoist_extra_waits(nc, inst, "split_wait")
                for nop in nops:
                    if cur_list and cur_list[-1] is nop:
                        cur_list.pop()
                    else:
                        cur_list.remove(nop)
                new_list.extend(nops)
                new_list.append(inst)
            bb.instructions[:] = new_list


def _drain_and_barrier(self, tick_clock, wait_clock):
    nc = self.nc
    _split_waits(nc)
    drain_inst = nc.sync.drain()
    wait_clock.add_sem_waits(drain_inst.ins,
                             ScopedClock({None: tick_clock.global_clock}))
    nops = _hoist_extra_waits(nc, drain_inst.ins, "drain_split_wait")
    if nops:
        insts = nc.cur_bb.bb.instructions
        di = insts.index(drain_inst.ins)
        insts.append(insts.pop(di))
    nc.all_engine_barrier()
    assert self.sems is not None
    popped = nc._tile_sem_poison_stack.pop()
    assert popped is self._sem_poison
    nc.clear_and_free_semaphores(list(self.sems.allocated().values()))
    nc.all_engine_barrier()


tile.TileContext._drain_and_barrier = _drain_and_barrier

# ---------------------------------------------------------------------------
# Program builder
# ---------------------------------------------------------------------------
dt = mybir.dt
F32 = dt.float32
F32R = dt.float32r

HPC = 4                     # heads per core
B = 512
H = W = 64
BSTRIDE = HPC * H * W
CSTRIDE = H * W
NG = 32                     # batch groups of 16
N_CORES = 8


def build(mm_f32r=True, ns_iters=15, c0=1.0 / 300.0,
          in_bufs=8, out_bufs=3, mid_bufs=2):
    nc = bass.Bass("TRN2", target_bir_lowering=False, debug=False,
                   num_devices=N_CORES)
    mmdt = F32R if mm_f32r else F32
    wkv = nc.dram_tensor("wkv", [B, HPC, H, W], mmdt, kind="ExternalInput")
    p_left = nc.dram_tensor("p_left", [HPC, H, H], F32, kind="ExternalInput")
    p_right = nc.dram_tensor("p_right", [HPC, W, W], F32,
                             kind="ExternalInput")
    out = nc.dram_tensor("out_scr", [NG, 128, 2048], F32,
                         kind="ExternalOutput")

    with tile.TileContext(nc) as tc, ExitStack() as ctx:
        const_pool = ctx.enter_context(tc.tile_pool(name="const", bufs=1))
        bd_pool = ctx.enter_context(tc.tile_pool(name="bd", bufs=1))

        ident = const_pool.tile([128, 128], F32, tag="ident")
        masks.make_identity(nc, ident[:])
        i64 = ident[0:64, 0:64]
        zeros = const_pool.tile([128, 128], F32, tag="zeros")
        nc.gpsimd.memset(zeros[:], 0.0)

        bdl = {}
        for P in range(2):
            for s in range(2):
                t = bd_pool.tile([128, 128], mmdt, tag=f"bdl{P}{s}")
                nc.vector.tensor_copy(t[:], zeros[:])
                bdl[(P, s)] = t
        bdr = []
        for c in range(HPC):
            t = bd_pool.tile([128, 128], mmdt, tag=f"bdr{c}")
            nc.vector.tensor_copy(t[:], zeros[:])
            bdr.append(t)

        # ---------------- Phase 1: Newton-Schulz Cayley ----------------
        with ExitStack() as nsctx:
            ns_sb = nsctx.enter_context(tc.tile_pool(name="ns_sb", bufs=2))
            ns_keep = nsctx.enter_context(tc.tile_pool(name="ns_keep",
                                                       bufs=2))
            ns_ps = nsctx.enter_context(
                tc.tile_pool(name="ns_ps", bufs=1, space="PSUM"))

            xs, xts, cts, ms = [], [], [], []
            for m in range(2 * HPC):
                side, c = divmod(m, HPC)
                src = p_left if side == 0 else p_right

                psb = ns_sb.tile([64, 64], F32, tag="p_in")
                nc.sync.dma_start(psb[:], src.ap()[c])

                ptp = ns_ps.tile([64, 64], F32, tag=f"nsp{m}")
                nc.tensor.transpose(ptp[:], psb[:], i64)

                asb = ns_keep.tile([64, 64], F32, tag=f"a{m}")
                nc.vector.tensor_sub(asb[:], psb[:], ptp[:])
                nc.vector.tensor_scalar_mul(asb[:], asb[:], 0.5)   # A

                bps = ns_ps.tile([64, 64], F32, tag=f"nsp{m}")
                nc.tensor.matmul(bps[:], asb[:], asb[:])           # B = A^T A
                msb = ns_keep.tile([64, 64], F32, tag=f"m{m}")
                nc.vector.tensor_add(msb[:], bps[:], i64)          # M = I + B

                ctsb = ns_keep.tile([64, 64], F32, tag=f"ct{m}")
                nc.vector.scalar_tensor_tensor(                    # 2A - B
                    ctsb[:], asb[:], 2.0, bps[:],
                    op0=mybir.AluOpType.mult, op1=mybir.AluOpType.subtract)
                nc.vector.tensor_add(ctsb[:], ctsb[:], i64)        # C^T

                xsb = ns_keep.tile([64, 64], F32, tag=f"x{m}")
                nc.vector.tensor_scalar_mul(xsb[:], i64, c0)       # X0
                xtsb = ns_keep.tile([64, 64], F32, tag=f"xt{m}")
                nc.vector.tensor_scalar_mul(xtsb[:], i64, c0)

                xs.append(xsb)
                xts.append(xtsb)
                cts.append(ctsb)
                ms.append(msb)

            for k in range(ns_iters):
                for m in range(2 * HPC):
                    ups = ns_ps.tile([64, 64], F32, tag=f"nsp{m}")
                    nc.tensor.matmul(ups[:], ms[m][:], xs[m][:])   # U = M X
                    usb = ns_sb.tile([64, 64], F32, tag=f"ns_u{m}")
                    if m % 2 == 0:
                        nc.vector.tensor_copy(usb[:], ups[:])
                    else:
                        nc.scalar.copy(usb[:], ups[:])
                    wps = ns_ps.tile([64, 64], F32, tag=f"nsp{m}")
                    nc.tensor.matmul(wps[:], xts[m][:], usb[:])    # W = X U
                    xnew = ns_keep.tile([64, 64], F32, tag=f"x{m}")
                    nc.vector.scalar_tensor_tensor(                # 2X - W
                        xnew[:], xs[m][:], 2.0, wps[:],
                        op0=mybir.AluOpType.mult,
                        op1=mybir.AluOpType.subtract)
                    xs[m] = xnew
                    xtps = ns_ps.tile([64, 64], F32, tag=f"nsp{m}")
                    nc.tensor.transpose(xtps[:], xnew[:], i64)
                    xtnew = ns_keep.tile([64, 64], F32, tag=f"xt{m}")
                    if m % 2 == 0:
                        nc.scalar.copy(xtnew[:], xtps[:])
                    else:
                        nc.vector.tensor_copy(xtnew[:], xtps[:])
                    xts[m] = xtnew

            for c in range(HPC):
                r1ps = ns_ps.tile([64, 64], F32, tag=f"nsp{c}")
                nc.tensor.matmul(r1ps[:], cts[c][:], xs[c][:])     # R1 = C X
                r1sb = ns_sb.tile([64, 64], F32, tag=f"r1_{c}")
                nc.vector.tensor_copy(r1sb[:], r1ps[:])
                P, hh = divmod(c, 2)
                for s in range(2):
                    # (R1[:, s::2])^T = parity-s rows of R1^T  -> [32, 64]
                    sl = bass.AP(r1sb.tensor, r1sb.offset + s,
                                 [list(r1sb.ap[0]), [2, 32]])
                    tps = ns_ps.tile([32, 64], F32, tag=f"nsp{c}")
                    nc.tensor.transpose(tps[:], sl, i64)
                    dst = bdl[(P, s)]
                    nc.vector.tensor_copy(
                        dst[64 * P + 32 * hh:64 * P + 32 * hh + 32,
                            64 * hh:64 * hh + 64], tps[:])

                mr = HPC + c
                r2ps = ns_ps.tile([64, 64], F32, tag=f"nsp{mr}")
                nc.tensor.matmul(r2ps[:], cts[mr][:], xs[mr][:])   # R2 = C X
                nc.vector.tensor_copy(bdr[c][0:64, 0:64], r2ps[:])
                nc.vector.tensor_copy(bdr[c][64:128, 64:128], r2ps[:])

        # ---------------- Phase 2: main loop ----------------
        io_pool = ctx.enter_context(tc.tile_pool(name="io", bufs=in_bufs))
        out_pool = ctx.enter_context(tc.tile_pool(name="outp", bufs=out_bufs))
        mid_pool = ctx.enter_context(tc.tile_pool(name="mid", bufs=mid_bufs))
        ps_pool = ctx.enter_context(
            tc.tile_pool(name="mainps", bufs=1, space="PSUM"))

        for g in range(NG):
            xin = io_pool.tile([128, 2048], mmdt, tag="xin")
            nc.sync.dma_start(
                xin[:], bass.AP(wkv, 16 * g * BSTRIDE,
                                [[128, 128], [BSTRIDE, 16], [1, 128]]))

            ysb = [mid_pool.tile([128, 1024], F32, tag=f"ysb{P}",
                                 name=f"ysb{P}_{g}") for P in range(2)]
            for half in range(2):
                for P in range(2):
                    yps = ps_pool.tile([128, 512], F32, tag=f"mm1_{P}",
                                       bufs=2)
                    for s in range(2):
                        base = xin[64 * P:64 * P + 64,
                                   1024 * half + 64 * s:
                                   1024 * half + 64 * s + 64]
                        rhs = bass.AP(base.tensor, base.offset,
                                      [list(base.ap[0]), [128, 8], [1, 64]])
                        nc.tensor.matmul(
                            yps[:], bdl[(P, s)][64 * P:64 * P + 64, :], rhs,
                            start=(s == 0), stop=(s == 1),
                            tile_position=(64 * P, 0))
                    dstv = ysb[P][:, 512 * half:512 * half + 512]
                    if (half + P) % 2 == 0:
                        nc.vector.tensor_copy(dstv, yps[:])
                    else:
                        nc.scalar.copy(dstv, yps[:])

            ytsb = [mid_pool.tile([128, 1024], mmdt, tag=f"ytsb{P}",
                                  name=f"ytsb{P}_{g}") for P in range(2)]
            for P in range(2):
                for hp in range(2):
                    tps = ps_pool.tile([128, 512], F32, tag="t1", bufs=2)
                    for q in range(4):
                        qq = 4 * hp + q
                        nc.tensor.transpose(
                            tps[:, 128 * q:128 * q + 128],
                            ysb[P][:, 128 * qq:128 * qq + 128], ident[:])
                    dstv = ytsb[P][:, 512 * hp:512 * hp + 512]
                    if (P + hp) % 2 == 0:
                        nc.vector.tensor_copy(dstv, tps[:])
                    else:
                        nc.scalar.copy(dstv, tps[:])

            zsb = out_pool.tile([128, 2048], F32, tag="zsb")
            for c in range(HPC):
                P, hh = divmod(c, 2)
                zps = ps_pool.tile([128, 512], F32, tag="mm2", bufs=2)
                base = ytsb[P][:, 64 * hh:64 * hh + 64]
                rhs = bass.AP(base.tensor, base.offset,
                              [list(base.ap[0]), [128, 8], [1, 64]])
                nc.tensor.matmul(zps[:], bdr[c][:], rhs)
                dstv = zsb[:, 512 * c:512 * c + 512]
                if c % 2 == 0:
                    nc.vector.tensor_copy(dstv, zps[:])
                else:
                    nc.scalar.copy(dstv, zps[:])

            nc.sync.dma_start(
                bass.AP(out, g * 128 * 2048, [[2048, 128], [1, 2048]]),
                zsb[:])

    return nc


def _unscramble(scr):
    """scr [NG, 128, 2048] -> [512, 4, 64, 64].
    scr[g, 64*bp + j, 512*h + 64*q + i] = Z[16g + 2q + bp, h][i, j]."""
    a = scr.reshape(NG, 2, 64, HPC, 8, 64)      # g, bp, j, h, q, i
    a = a.transpose(0, 4, 1, 3, 5, 2)           # g, q, bp, h, i, j
    return np.ascontiguousarray(a.reshape(B, HPC, H, W))


_CACHED = {}


def _get_program():
    if "nc" not in _CACHED:
        _CACHED["nc"] = build()
    return _CACHED["nc"]


def kernel(wkv, p_left, p_right):
    wkv = np.ascontiguousarray(wkv, dtype=np.float32)
    p_left = np.ascontiguousarray(p_left, dtype=np.float32)
    p_right = np.ascontiguousarray(p_right, dtype=np.float32)
    assert wkv.shape == (B, 32, H, W), wkv.shape

    nc = _get_program()
    in_maps = []
    for k in range(N_CORES):
        sl = slice(HPC * k, HPC * k + HPC)
        in_maps.append({
            "wkv": np.ascontiguousarray(wkv[:, sl]),
            "p_left": np.ascontiguousarray(p_left[sl]),
            "p_right": np.ascontiguousarray(p_right[sl]),
        })
    res = run_bass_kernel_spmd(nc, in_maps, list(range(N_CORES)))
    return np.concatenate(
        [_unscramble(np.asarray(res.results[k]["out_scr"]))
         for k in range(N_CORES)], axis=1)



# revision 2
# speedup vs baseline: 1.4966x; 1.4966x over previous
"""BilateralRotation Trainium2 kernel: out[b,c] = R1[c] @ wkv[b,c] @ R2[c],
R = Cayley(p) = (I - A)(I + A)^-1, A = 0.5(p - p^T).

Sharding: 8 NeuronCores, head-parallel — core k owns heads [4k, 4k+4) for all
512 batches (32 MB in / 32 MB out per core).

The tiny per-head rotations R1/R2 are computed on the HOST (fp64 numpy) and
shipped to each core pre-packed as the exact 128x128 block-diagonal stationary
matrices the device matmuls consume (bdl for MM1 parity-split, bdr for MM2).
This removes the on-device Newton-Schulz phase that previously serialized
~115us of pure compute in front of the DMA pipeline.

Device program per core (pure streaming):
  - input DMA in a folded layout (each partition holds two consecutive
    h-rows = 512B contiguous elements; 128 partitions span the core's 4
    heads with a single affine stride)
  - MM1 (Y = R1 X): two parity-split accumulating matmuls per head-pair,
    stationary = blockdiag of parity-sliced R1^T, moving = data, float32r
  - T1: PE 128x128 transposes (4 items each)
  - MM2 (Z^T = R2^T Y^T): stationary blockdiag(R2,R2), strided rhs
    gathering one head's columns, N=512, float32r
  - Z^T tiles dumped contiguously to HBM; the host inverts the (fixed,
    known) index permutation while unsharding/concatenating the 8 shards.
"""

import sys
import types
from contextlib import ExitStack

import numpy as np

# ---------------------------------------------------------------------------
# TileContext patch: this walrus build accepts only ONE sync-wait per
# instruction; hoist extra waits onto nops inserted before the instruction.
# ---------------------------------------------------------------------------
import concourse.bass as bass
import concourse.tile as tile
from concourse.vector_clock import ScopedClock
from concourse import masks, mybir
from concourse.bass_utils import run_bass_kernel_spmd

WAIT_LIMIT = 1


def _hoist_extra_waits(nc, inst, hint):
    nops = []
    si = inst.sync_info
    if si is not None and len(si.on_wait) > WAIT_LIMIT:
        extras = si.on_wait[:-WAIT_LIMIT]
        del si.on_wait[:-WAIT_LIMIT]
        for w in extras:
            nop = nc.engines[inst.engine].nop(nofuse=True, hint=hint)
            nsi = nop.ins.sync_info
            if nsi is None:
                nop.ins.sync_info = mybir.SyncInfo(on_wait=[w], on_update=[])
            else:
                nsi.on_wait.append(w)
            nops.append(nop.ins)
    return nops


def _split_waits(nc):
    cur_list = nc.cur_bb.bb.instructions
    for f in nc.m.functions:
        for bb in f.blocks:
            orig = list(bb.instructions)
            if not any(i.sync_info and len(i.sync_info.on_wait) > WAIT_LIMIT
                       for i in orig):
                continue
            new_list = []
            for inst in orig:
                nops = _hoist_extra_waits(nc, inst, "split_wait")
                for nop in nops:
                    if cur_list and cur_list[-1] is nop:
                        cur_list.pop()
                    else:
                        cur_list.remove(nop)
                new_list.extend(nops)
                new_list.append(inst)
            bb.instructions[:] = new_list


def _drain_and_barrier(self, tick_clock, wait_clock):
    nc = self.nc
    _split_waits(nc)
    drain_inst = nc.sync.drain()
    wait_clock.add_sem_waits(drain_inst.ins,
                             ScopedClock({None: tick_clock.global_clock}))
    nops = _hoist_extra_waits(nc, drain_inst.ins, "drain_split_wait")
    if nops:
        insts = nc.cur_bb.bb.instructions
        di = insts.index(drain_inst.ins)
        insts.append(insts.pop(di))
    nc.all_engine_barrier()
    assert self.sems is not None
    popped = nc._tile_sem_poison_stack.pop()
    assert popped is self._sem_poison
    nc.clear_and_free_semaphores(list(self.sems.allocated().values()))
    nc.all_engine_barrier()


tile.TileContext._drain_and_barrier = _drain_and_barrier

# ---------------------------------------------------------------------------
# Program builder
# ---------------------------------------------------------------------------
dt = mybir.dt
F32 = dt.float32
F32R = dt.float32r

HPC = 4                     # heads per core
B = 512
H = W = 64
BSTRIDE = HPC * H * W
CSTRIDE = H * W
NG = 32                     # batch groups of 16
N_CORES = 8


def build(mm_f32r=True, in_bufs=8, out_bufs=3, mid_bufs=2):
    nc = bass.Bass("TRN2", target_bir_lowering=False, debug=False,
                   num_devices=N_CORES)
    mmdt = F32R if mm_f32r else F32
    wkv = nc.dram_tensor("wkv", [B, HPC, H, W], mmdt, kind="ExternalInput")
    bdl_d = nc.dram_tensor("bdl", [2, 2, 128, 128], mmdt,
                           kind="ExternalInput")
    bdr_d = nc.dram_tensor("bdr", [HPC, 128, 128], mmdt,
                           kind="ExternalInput")
    out = nc.dram_tensor("out_scr", [NG, 128, 2048], F32,
                         kind="ExternalOutput")

    with tile.TileContext(nc) as tc, ExitStack() as ctx:
        const_pool = ctx.enter_context(tc.tile_pool(name="const", bufs=1))
        bd_pool = ctx.enter_context(tc.tile_pool(name="bd", bufs=1))

        ident = const_pool.tile([128, 128], F32, tag="ident")
        masks.make_identity(nc, ident[:])

        # Host-precomputed stationaries: MM1 parity blockdiags + MM2
        # head blockdiags. 8 x 64KB DMAs, negligible next to the 32MB stream.
        bdl = {}
        for P in range(2):
            for s in range(2):
                t = bd_pool.tile([128, 128], mmdt, tag=f"bdl{P}{s}")
                nc.sync.dma_start(t[:], bdl_d.ap()[P][s])
                bdl[(P, s)] = t
        bdr = []
        for c in range(HPC):
            t = bd_pool.tile([128, 128], mmdt, tag=f"bdr{c}")
            nc.sync.dma_start(t[:], bdr_d.ap()[c])
            bdr.append(t)

        # ---------------- main loop ----------------
        io_pool = ctx.enter_context(tc.tile_pool(name="io", bufs=in_bufs))
        out_pool = ctx.enter_context(tc.tile_pool(name="outp", bufs=out_bufs))
        mid_pool = ctx.enter_context(tc.tile_pool(name="mid", bufs=mid_bufs))
        ps_pool = ctx.enter_context(
            tc.tile_pool(name="mainps", bufs=1, space="PSUM"))

        for g in range(NG):
            xin = io_pool.tile([128, 2048], mmdt, tag="xin")
            nc.sync.dma_start(
                xin[:], bass.AP(wkv, 16 * g * BSTRIDE,
                                [[128, 128], [BSTRIDE, 16], [1, 128]]))

            ysb = [mid_pool.tile([128, 1024], F32, tag=f"ysb{P}",
                                 name=f"ysb{P}_{g}") for P in range(2)]
            for half in range(2):
                for P in range(2):
                    yps = ps_pool.tile([128, 512], F32, tag=f"mm1_{P}",
                                       bufs=2)
                    for s in range(2):
                        base = xin[64 * P:64 * P + 64,
                                   1024 * half + 64 * s:
                                   1024 * half + 64 * s + 64]
                        rhs = bass.AP(base.tensor, base.offset,
                                      [list(base.ap[0]), [128, 8], [1, 64]])
                        nc.tensor.matmul(
                            yps[:], bdl[(P, s)][64 * P:64 * P + 64, :], rhs,
                            start=(s == 0), stop=(s == 1),
                            tile_position=(64 * P, 0))
                    dstv = ysb[P][:, 512 * half:512 * half + 512]
                    if (half + P) % 2 == 0:
                        nc.vector.tensor_copy(dstv, yps[:])
                    else:
                        nc.scalar.copy(dstv, yps[:])

            ytsb = [mid_pool.tile([128, 1024], mmdt, tag=f"ytsb{P}",
                                  name=f"ytsb{P}_{g}") for P in range(2)]
            for P in range(2):
                for hp in range(2):
                    tps = ps_pool.tile([128, 512], F32, tag="t1", bufs=2)
                    for q in range(4):
                        qq = 4 * hp + q
                        nc.tensor.transpose(
                            tps[:, 128 * q:128 * q + 128],
                            ysb[P][:, 128 * qq:128 * qq + 128], ident[:])
                    dstv = ytsb[P][:, 512 * hp:512 * hp + 512]
                    if (P + hp) % 2 == 0:
                        nc.vector.tensor_copy(dstv, tps[:])
                    else:
                        nc.scalar.copy(dstv, tps[:])

            zsb = out_pool.tile([128, 2048], F32, tag="zsb")
            for c in range(HPC):
                P, hh = divmod(c, 2)
                zps = ps_pool.tile([128, 512], F32, tag="mm2", bufs=2)
                base = ytsb[P][:, 64 * hh:64 * hh + 64]
                rhs = bass.AP(base.tensor, base.offset,
                              [list(base.ap[0]), [128, 8], [1, 64]])
                nc.tensor.matmul(zps[:], bdr[c][:], rhs)
                dstv = zsb[:, 512 * c:512 * c + 512]
                if c % 2 == 0:
                    nc.vector.tensor_copy(dstv, zps[:])
                else:
                    nc.scalar.copy(dstv, zps[:])

            nc.sync.dma_start(
                bass.AP(out, g * 128 * 2048, [[2048, 128], [1, 2048]]),
                zsb[:])

    return nc


# ---------------------------------------------------------------------------
# Host-side rotation precompute
# ---------------------------------------------------------------------------
def _cayley_np(p):
    """R = (I - A)(I + A)^-1, A = 0.5(p - p^T); fp64 for exactness."""
    p = p.astype(np.float64)
    a = 0.5 * (p - np.swapaxes(p, -1, -2))
    eye = np.eye(p.shape[-1])
    inv = np.linalg.solve(eye[None] + a, np.broadcast_to(eye, a.shape))
    return (eye[None] - a) @ inv


def _pack_bd(r1, r2):
    """r1, r2: [HPC, 64, 64] fp32 -> (bdl [2,2,128,128], bdr [HPC,128,128]).

    bdl[P, s]: rows 64P+32hh..+32 x cols 64hh..+64 hold (R1[c][:, s::2])^T
    for c = 2P + hh — the parity-split MM1 stationary (stationary slice
    [64P:64P+64, :] maps moving partition (c, rowpair) -> output (hh, i)).
    bdr[c] = blockdiag(R2[c], R2[c]) for the N=512 MM2."""
    bdl = np.zeros((2, 2, 128, 128), dtype=np.float32)
    bdr = np.zeros((HPC, 128, 128), dtype=np.float32)
    for c in range(HPC):
        P, hh = divmod(c, 2)
        for s in range(2):
            blk = r1[c][:, s::2].T          # [32, 64] = [k2, i]
            r0 = 64 * P + 32 * hh
            bdl[P, s, r0:r0 + 32, 64 * hh:64 * hh + 64] = blk
        bdr[c, 0:64, 0:64] = r2[c]
        bdr[c, 64:128, 64:128] = r2[c]
    return bdl, bdr


def _unscramble(scr):
    """scr [NG, 128, 2048] -> [512, 4, 64, 64].
    scr[g, 64*bp + j, 512*h + 64*q + i] = Z[16g + 2q + bp, h][i, j]."""
    a = scr.reshape(NG, 2, 64, HPC, 8, 64)      # g, bp, j, h, q, i
    a = a.transpose(0, 4, 1, 3, 5, 2)           # g, q, bp, h, i, j
    return np.ascontiguousarray(a.reshape(B, HPC, H, W))


_CACHED = {}


def _get_program():
    if "nc" not in _CACHED:
        _CACHED["nc"] = build()
    return _CACHED["nc"]


def kernel(wkv, p_left, p_right):
    wkv = np.ascontiguousarray(wkv, dtype=np.float32)
    p_left = np.ascontiguousarray(p_left, dtype=np.float32)
    p_right = np.ascontiguousarray(p_right, dtype=np.float32)
    assert wkv.shape == (B, 32, H, W), wkv.shape

    r1_all = _cayley_np(p_left).astype(np.float32)    # [32, 64, 64]
    r2_all = _cayley_np(p_right).astype(np.float32)   # [32, 64, 64]

    nc = _get_program()
    in_maps = []
    for k in range(N_CORES):
        sl = slice(HPC * k, HPC * k + HPC)
        bdl, bdr = _pack_bd(r1_all[sl], r2_all[sl])
        in_maps.append({
            "wkv": np.ascontiguousarray(wkv[:, sl]),
            "bdl": bdl,
            "bdr": bdr,
        })
    res = run_bass_kernel_spmd(nc, in_maps, list(range(N_CORES)))
    return np.concatenate(
        [_unscramble(np.asarray(res.results[k]["out_scr"]))
         for k in range(N_CORES)], axis=1)
